# revision 3
# baseline (speedup 1.0000x reference)
"""LSTM cell (B=4096, D=U=2048) on 8 trn2 NeuronCores — mixed fp8/bf16.

Tensor-parallel over units (core i owns units [i*256,(i+1)*256) of every
gate). Per-gate mixed precision exploits the LSTM's uneven error
sensitivities: the i gate (and most of f) run their GEMMs in e4m3 with
DoubleRow pairs at the fp8 double-pump rate, while the error-critical g
(tanh) and o gates stay in bf16. All weights are pre-scaled by 64 (exact in
bf16, keeps e4m3 out of denormals) so fp8 and bf16 matmuls accumulate into
the same PSUM bank at a common scale; the 1/64 dequant is fused into the
gate activation.

Measured on the reference inputs: h rel err ~1.7e-2, c ~1.7e-2 (gate 2e-2).
"""

import sys

sys.path.insert(0, "/opt/trn_rl_repo")

import ml_dtypes
import numpy as np

import concourse.bass as bass
import concourse.mybir as mybir
import concourse.tile as tile
from concourse.bass_utils import run_bass_kernel_spmd

B, D, U = 4096, 2048, 2048
N_CORES = 8
US = U // N_CORES          # units per core per gate (256)
UT = US // 128             # unit tiles of 128 per gate (2)
NB = 512                   # batch tile (free dim)
NT = B // NB               # batch tiles (8)
KX = D // 128              # k tiles per gemm (16)
PAIRS = KX // 2            # DoubleRow k-pairs per gemm (8)
NP8_F = 8                  # k-pairs (of 8) of gate f in fp8 (all)
NP8_I = 8                  # k-pairs (of 8) of gate i in fp8
W_SCALE = 64.0
F8 = mybir.dt.float8e4
BF16 = mybir.dt.bfloat16
F32 = mybir.dt.float32
AF = mybir.ActivationFunctionType
DR = mybir.MatmulPerfMode.DoubleRow


def _split_excess_waits(nc, maxw=1):
    """This walrus build rejects instructions carrying more than one sem-wait
    ("Too many sync wait commands"), but Tile freely attaches several. Hoist
    the extra waits onto same-engine nops inserted right before the
    instruction — engine streams are in-order, so blocking semantics are
    identical."""
    cnt = 0
    for fn in nc.m.functions:
        for bb in fn.blocks:
            new_insts = []
            for inst in bb.instructions:
                si = inst.sync_info
                waits = list(si.on_wait) if si is not None else []
                if len(waits) > maxw:
                    for i in range(0, len(waits) - maxw, maxw):
                        nop = mybir.InstNoOp(name=f"syncsplit-{cnt}")
                        cnt += 1
                        nop.engine = inst.engine
                        nop.sync_info = mybir.SyncInfo(
                            on_wait=waits[i : i + maxw], on_update=[]
                        )
                        new_insts.append(nop)
                    si.on_wait = waits[len(waits) - maxw :]
                new_insts.append(inst)
            if len(new_insts) != len(bb.instructions):
                bb.instructions = new_insts
    return cnt


def build_nc() -> bass.Bass:
    nc = bass.Bass()

    xT8 = nc.dram_tensor("xT8", [D, B], F8, kind="ExternalInput")
    hT8 = nc.dram_tensor("hT8", [U, B], F8, kind="ExternalInput")
    xTb = nc.dram_tensor("xTb", [D, B], BF16, kind="ExternalInput")
    hTb = nc.dram_tensor("hTb", [U, B], BF16, kind="ExternalInput")
    # fp8 weights: gates [f, i] only, 64x pre-scale   [D, 512]
    wx8 = nc.dram_tensor("wx8", [D, 2 * US], F8, kind="ExternalInput")
    wh8 = nc.dram_tensor("wh8", [U, 2 * US], F8, kind="ExternalInput")
    # bf16 weights: all gates [f,i,o,g], 64x pre-scale [D, 1024]; only the
    # o,g columns and f's k-tail rows are ever read
    wxb = nc.dram_tensor("wxb", [D, 4 * US], BF16, kind="ExternalInput")
    whb = nc.dram_tensor("whb", [U, 4 * US], BF16, kind="ExternalInput")
    # bias, host-prepped to [128, 8]: column j = units [j*128,(j+1)*128) of
    # the concatenated [f,i,o,g] 1024-unit block (gate j//2, unit-tile j%2)
    bias = nc.dram_tensor("bias", [128, 4 * UT], F32, kind="ExternalInput")
    cT = nc.dram_tensor("cT", [US, B], F32, kind="ExternalInput")
    h_newT = nc.dram_tensor("h_newT", [US, B], F32, kind="ExternalOutput")
    c_newT = nc.dram_tensor("c_newT", [US, B], F32, kind="ExternalOutput")

    wx8_r = wx8.rearrange("(kp t p) u -> p kp t u", p=128, t=2)  # [128,8,2,512]
    wh8_r = wh8.rearrange("(kp t p) u -> p kp t u", p=128, t=2)
    wxb_r = wxb.rearrange("(kt p) u -> p kt u", p=128)           # [128,16,1024]
    whb_r = whb.rearrange("(kt p) u -> p kt u", p=128)
    xT8_r = xT8.rearrange("(kt p) b -> p kt b", p=128)           # [128,16,B]
    hT8_r = hT8.rearrange("(kt p) b -> p kt b", p=128)
    xTb_r = xTb.rearrange("(kt p) b -> p kt b", p=128)
    hTb_r = hTb.rearrange("(kt p) b -> p kt b", p=128)

    with tile.TileContext(nc) as tc:
        with (
            tc.tile_pool(name="wpool", bufs=1) as wpool,
            tc.tile_pool(name="singles", bufs=1) as singles,
            tc.tile_pool(name="acts", bufs=2) as apool,
            tc.tile_pool(name="ew", bufs=3) as epool,
            tc.tile_pool(name="psum", bufs=8, space="PSUM") as ppool,
        ):
            nsl0 = bass.ts(0, NB)
            # ---- startup stream (single FIFO ring), ordered to track the
            # n=0 k-outer compute: fp8 acts+weights first (gates i,f), then
            # the bf16 side (gates o,g and f's k-tail).
            # n=0 activations arrive chunked; the chunk tiles share the
            # steady-state pool tags, so the bufs=2 rotation both bounds SBUF
            # and threads the n=1 reuse dependencies. The first x/weight
            # chunks are small so the first DoubleRow matmul starts early;
            # weights use separate tiles per chunk for fine-grained deps.
            x80c = []
            for k0, k1 in ((0, 4), (4, 8), (8, 16)):
                # first chunk gets a dedicated slot so the shared x8_sb tag
                # sees only 2 n=0 allocations (no FIFO-stalling WAR reuse)
                tag = "x80a" if k0 == 0 else "x8_sb"
                t = apool.tile([128, k1 - k0, NB], F8, tag=tag, bufs=1 if k0 == 0 else None)
                nc.sync.dma_start(out=t[:], in_=xT8_r[:, k0:k1, nsl0])
                x80c.append((k0, t))
                if k0 == 0:
                    wx8_a = wpool.tile([128, 2, 2, 2 * US], F8)
                    nc.sync.dma_start(out=wx8_a[:], in_=wx8_r[:, 0:2])
                if k0 == 4:
                    wx8_b = wpool.tile([128, PAIRS - 2, 2, 2 * US], F8)
                    nc.sync.dma_start(out=wx8_b[:], in_=wx8_r[:, 2:])

            def x80(j):
                ci = min(j // 2, 2)
                k0, t = x80c[ci]
                return t[:, 2 * j - k0 : 2 * j - k0 + 2, :]
            h80c = []
            xb0 = []
            wxb_og_c = []

            def h8_chunk(jc):
                t = apool.tile([128, 8, NB], F8, tag="h8_sb", name=f"h80c{jc}")
                nc.sync.dma_start(out=t[:], in_=hT8_r[:, 8 * jc : 8 * jc + 8, nsl0])
                h80c.append(t)
                wt = wpool.tile([128, PAIRS // 2, 2, 2 * US], F8, tag=f"wh8{jc}")
                nc.sync.dma_start(out=wt[:], in_=wh8_r[:, 4 * jc : 4 * jc + 4])
                return wt

            def xb_chunk(jc):
                t = apool.tile([128, 8, NB], BF16, tag="xb_sb", name=f"xb0c{jc}")
                nc.sync.dma_start(out=t[:], in_=xTb_r[:, 8 * jc : 8 * jc + 8, nsl0])
                xb0.append(t)
                wt = wpool.tile([128, 8, 2 * US], BF16, tag=f"wxog{jc}")
                nc.sync.dma_start(out=wt[:], in_=wxb_r[:, 8 * jc : 8 * jc + 8, 2 * US :])
                wxb_og_c.append(wt)

            wh8_a = h8_chunk(0)
            xb_chunk(0)
            wh8_b = h8_chunk(1)
            b_sb = singles.tile([128, 4 * UT], F32)
            nc.sync.dma_start(out=b_sb[:], in_=bias[:])
            xb_chunk(1)
            h80 = lambda j: h80c[j // 4][:, 2 * j - 8 * (j // 4) : 2 * j - 8 * (j // 4) + 2, :]
            hb0 = []
            whb_og_c = []
            for jc in range(2):
                t = apool.tile([128, 8, NB], BF16, tag="hb_sb")
                nc.sync.dma_start(out=t[:], in_=hTb_r[:, 8 * jc : 8 * jc + 8, nsl0])
                hb0.append(t)
                wt = wpool.tile([128, 8, 2 * US], BF16, tag=f"whog{jc}")
                nc.sync.dma_start(out=wt[:], in_=whb_r[:, 8 * jc : 8 * jc + 8, 2 * US :])
                whb_og_c.append(wt)

            def act_gate(ps, gi, ut, name):
                g_sb = epool.tile([128, NB], F32, tag=f"gate{gi}", name=name)
                nc.scalar.activation(
                    g_sb[:],
                    ps[:],
                    AF.Tanh if gi == 3 else AF.Sigmoid,
                    bias=b_sb[:, gi * UT + ut : gi * UT + ut + 1],
                    scale=1.0 / W_SCALE,
                )
                return g_sb

            def elementwise(pss, n, ut):
                # pss indexed by gate order [f, i, o, g]; consume gates in
                # completion order g, i, f, o
                nsl = bass.ts(n, NB)
                usl = slice(ut * 128, (ut + 1) * 128)
                c_sb = epool.tile([128, NB], F32, tag="c_sb", name="c_sb")
                nc.sync.dma_start(out=c_sb[:], in_=cT[usl, nsl])
                g_t = act_gate(pss[3], 3, ut, "g_t")
                i_t = act_gate(pss[1], 1, ut, "i_t")
                nc.vector.tensor_mul(i_t[:], i_t[:], g_t[:])      # i*g
                f_t = act_gate(pss[0], 0, ut, "f_t")
                nc.vector.tensor_mul(f_t[:], f_t[:], c_sb[:])     # f*c
                cn = epool.tile([128, NB], F32, tag="cn", name="cn")
                nc.vector.tensor_add(cn[:], f_t[:], i_t[:])       # c_new
                nc.sync.dma_start(out=c_newT[usl, nsl], in_=cn[:])
                nc.scalar.activation(g_t[:], cn[:], AF.Tanh)      # tanh(c_new)
                o_t = act_gate(pss[2], 2, ut, "o_t")
                nc.vector.tensor_mul(o_t[:], o_t[:], g_t[:])      # h_new
                nc.sync.dma_start(out=h_newT[usl, nsl], in_=o_t[:])

            # weight AP accessors over the chunked tiles; column base:
            # fp8 tiles hold gates [f, i], og tiles hold gates [o, g]
            def wx8_ap(j, gi, ut):
                t, jj = (wx8_a, j) if j < 2 else (wx8_b, j - 2)
                c = gi * US + ut * 128
                return t[:, jj, :, c : c + 128]

            def wh8_ap(j, gi, ut):
                t, jj = (wh8_a, j) if j < 4 else (wh8_b, j - 4)
                c = gi * US + ut * 128
                return t[:, jj, :, c : c + 128]

            def wxog_ap(kt, gi, ut):
                c = (gi - 2) * US + ut * 128
                return wxb_og_c[kt // 8][:, kt % 8, c : c + 128]

            def whog_ap(kt, gi, ut):
                c = (gi - 2) * US + ut * 128
                return whb_og_c[kt // 8][:, kt % 8, c : c + 128]

            # ---- matmul emitters (shared between n=0 k-outer and n>=1) ----
            def mm_f8(ps, w_ap, mv, start, stop):
                nc.tensor.matmul(
                    ps[:], w_ap, mv, start=start, stop=stop, perf_mode=DR
                )

            def mm_og(ps, w_ap, mv, start, stop):
                nc.tensor.matmul(ps[:], w_ap, mv, start=start, stop=stop)

            # --- n = 0: k-outer, ordered to track the DMA arrival stream.
            # Groups: (gate, ut) -> PSUM bank; i,f consume fp8 pairs first,
            # then o,g (+ f tail) consume the bf16 stream.
            ps0 = [
                [
                    ppool.tile([128, NB], F32, tag="ps", name=f"ps{ut}{gi}")
                    for gi in range(4)
                ]
                for ut in range(UT)
            ]
            # fp8 x pairs (i then f per pair so i leads)
            for j in range(PAIRS):
                mv = x80(j)
                for ut in range(UT):
                    mm_f8(ps0[ut][1], wx8_ap(j, 1, ut), mv, start=(j == 0), stop=False)
                    mm_f8(ps0[ut][0], wx8_ap(j, 0, ut), mv, start=(j == 0), stop=False)
            # fp8 h pairs
            for j in range(PAIRS):
                mv = h80(j)
                for ut in range(UT):
                    mm_f8(ps0[ut][1], wh8_ap(j, 1, ut), mv, start=False, stop=(j == PAIRS - 1))
                    mm_f8(ps0[ut][0], wh8_ap(j, 0, ut), mv, start=False, stop=(j == PAIRS - 1))
            # bf16 x k-tiles: o,g (+ f tail rows)
            for kt in range(KX):
                mv = xb0[kt // 8][:, kt % 8, :]
                for ut in range(UT):
                    for gi in (3, 2):
                        mm_og(ps0[ut][gi], wxog_ap(kt, gi, ut), mv, start=(kt == 0), stop=False)
            # bf16 h k-tiles
            for kt in range(KX):
                mv = hb0[kt // 8][:, kt % 8, :]
                for ut in range(UT):
                    for gi in (3, 2):
                        mm_og(ps0[ut][gi], whog_ap(kt, gi, ut), mv, start=False, stop=(kt == KX - 1))
            for ut in range(UT):
                elementwise(ps0[ut], 0, ut)

            # --- n = 1..7: gate-outer in consumption order (g, i, f, o)
            for n in range(1, NT):
                nsl = bass.ts(n, NB)
                x8_sb = apool.tile([128, KX, NB], F8, tag="x8_sb")
                nc.sync.dma_start(out=x8_sb[:], in_=xT8_r[:, :, nsl])
                h8_sb = apool.tile([128, KX, NB], F8, tag="h8_sb")
                nc.sync.dma_start(out=h8_sb[:], in_=hT8_r[:, :, nsl])
                xb_sb = apool.tile([128, KX, NB], BF16, tag="xb_sb")
                nc.sync.dma_start(out=xb_sb[:], in_=xTb_r[:, :, nsl])
                hb_sb = apool.tile([128, KX, NB], BF16, tag="hb_sb")
                nc.sync.dma_start(out=hb_sb[:], in_=hTb_r[:, :, nsl])

                for ut in range(UT):
                    pss = [
                        ppool.tile([128, NB], F32, tag="ps", name=f"ps{gi}")
                        for gi in range(4)
                    ]
                    # gate i first: its fp8 x tile is the iteration's
                    # earliest-arriving input
                    for j in range(PAIRS):
                        mm_f8(pss[1], wx8_ap(j, 1, ut), x8_sb[:, 2 * j : 2 * j + 2, :],
                              start=(j == 0), stop=False)
                    for j in range(PAIRS):
                        mm_f8(pss[1], wh8_ap(j, 1, ut), h8_sb[:, 2 * j : 2 * j + 2, :],
                              start=False, stop=(j == PAIRS - 1))
                    # gate g: all bf16
                    for kt in range(KX):
                        mm_og(pss[3], wxog_ap(kt, 3, ut), xb_sb[:, kt, :],
                              start=(kt == 0), stop=False)
                    for kt in range(KX):
                        mm_og(pss[3], whog_ap(kt, 3, ut), hb_sb[:, kt, :],
                              start=False, stop=(kt == KX - 1))
                    # gate f: all fp8 pairs
                    for j in range(PAIRS):
                        mm_f8(pss[0], wx8_ap(j, 0, ut), x8_sb[:, 2 * j : 2 * j + 2, :],
                              start=(j == 0), stop=False)
                    for j in range(PAIRS):
                        mm_f8(pss[0], wh8_ap(j, 0, ut), h8_sb[:, 2 * j : 2 * j + 2, :],
                              start=False, stop=(j == PAIRS - 1))
                    # gate o: all bf16
                    for kt in range(KX):
                        mm_og(pss[2], wxog_ap(kt, 2, ut), xb_sb[:, kt, :],
                              start=(kt == 0), stop=False)
                    for kt in range(KX):
                        mm_og(pss[2], whog_ap(kt, 2, ut), hb_sb[:, kt, :],
                              start=False, stop=(kt == KX - 1))
                    elementwise(pss, n, ut)
    _split_excess_waits(nc)
    return nc


_NC_CACHE = None


def _get_nc():
    global _NC_CACHE
    if _NC_CACHE is None:
        _NC_CACHE = build_nc()
    return _NC_CACHE


def make_in_maps(x, h, c, Wxf, Wxi, Wxo, Wxg, bf, bi, bo, bg, Whf, Whi, Who, Whg):
    f8 = ml_dtypes.float8_e4m3
    bf16 = ml_dtypes.bfloat16
    x = np.asarray(x, np.float32)
    h = np.asarray(h, np.float32)
    xT = np.ascontiguousarray(x.T)
    hT = np.ascontiguousarray(h.T)
    xT8 = xT.astype(f8)
    hT8 = hT.astype(f8)
    xTb = xT.astype(bf16)
    hTb = hT.astype(bf16)
    c = np.asarray(c, np.float32)
    Wx = np.stack([np.asarray(w, np.float32) for w in (Wxf, Wxi, Wxo, Wxg)])
    Wh = np.stack([np.asarray(w, np.float32) for w in (Whf, Whi, Who, Whg)])
    bias = np.stack([np.asarray(v, np.float32) for v in (bf, bi, bo, bg)])

    in_maps = []
    for i in range(N_CORES):
        s = slice(i * US, (i + 1) * US)
        wx8_i = np.concatenate(
            [Wx[g, :, s] * W_SCALE for g in range(2)], axis=1
        ).astype(f8)
        wh8_i = np.concatenate(
            [Wh[g, :, s] * W_SCALE for g in range(2)], axis=1
        ).astype(f8)
        wxb_i = np.concatenate(
            [Wx[g, :, s] * W_SCALE for g in range(4)], axis=1
        ).astype(bf16)
        whb_i = np.concatenate(
            [Wh[g, :, s] * W_SCALE for g in range(4)], axis=1
        ).astype(bf16)
        b_i = np.concatenate([bias[g, s] for g in range(4)])  # [1024]
        b_i = np.ascontiguousarray(b_i.reshape(4 * UT, 128).T)  # [128, 8]
        cT_i = np.ascontiguousarray(c[:, s].T)  # [US, B]
        in_maps.append(
            {
                "xT8": xT8, "hT8": hT8, "xTb": xTb, "hTb": hTb,
                "wx8": wx8_i, "wh8": wh8_i, "wxb": wxb_i, "whb": whb_i,
                "bias": b_i, "cT": cT_i,
            }
        )
    return in_maps


def run(in_maps, **kwargs):
    nc = _get_nc()
    return run_bass_kernel_spmd(nc, in_maps, list(range(N_CORES)), **kwargs)


def gather(results):
    h_new = np.empty((B, U), np.float32)
    c_new = np.empty((B, U), np.float32)
    for i in range(N_CORES):
        s = slice(i * US, (i + 1) * US)
        h_new[:, s] = results[i]["h_newT"].T
        c_new[:, s] = results[i]["c_newT"].T
    return h_new, c_new


def kernel(**inputs):
    res = run(make_in_maps(**inputs))
    return gather(res.results)


# revision 4
# speedup vs baseline: 1.2097x; 1.2097x over previous
"""LSTM cell (B=4096, D=U=2048) on 8 trn2 NeuronCores — mixed fp8/bf16.

Tensor-parallel over units (core i owns units [i*256,(i+1)*256) of every
gate). Per-gate mixed precision exploits the LSTM's uneven error
sensitivities: the f and i gates run their GEMMs entirely in e4m3 with
DoubleRow pairs at the fp8 double-pump rate (2x), while the error-critical g
(tanh' up to 1, feeds both outputs) and o gates stay in bf16. That removes
25% of PE cycles. All weights are pre-scaled by 64 (exact in bf16, keeps
e4m3 out of denormals) so fp8 and bf16 matmuls accumulate into the same
PSUM bank at a common scale; the 1/64 dequant is fused into the gate
activation's scale operand.

Measured on the reference inputs (deterministic, fixed jax key): h rel err
1.740e-2, c 1.813e-2 (gate 2e-2); traced HW exec 364-425us vs the bf16
baseline's 553us in the same conditions.
"""

import sys

sys.path.insert(0, "/opt/trn_rl_repo")

import ml_dtypes
import numpy as np

import concourse.bass as bass
import concourse.mybir as mybir
import concourse.tile as tile
from concourse.bass_utils import run_bass_kernel_spmd

B, D, U = 4096, 2048, 2048
N_CORES = 8
US = U // N_CORES          # units per core per gate (256)
UT = US // 128             # unit tiles of 128 per gate (2)
NB = 512                   # batch tile (free dim)
NT = B // NB               # batch tiles (8)
KX = D // 128              # k tiles per gemm (16)
PAIRS = KX // 2            # DoubleRow k-pairs per gemm (8)
NP8_F = 8                  # k-pairs (of 8) of gate f in fp8 (all)
NP8_I = 8                  # k-pairs (of 8) of gate i in fp8
W_SCALE = 64.0
F8 = mybir.dt.float8e4
BF16 = mybir.dt.bfloat16
F32 = mybir.dt.float32
AF = mybir.ActivationFunctionType
DR = mybir.MatmulPerfMode.DoubleRow


def _split_excess_waits(nc, maxw=1):
    """This walrus build rejects instructions carrying more than one sem-wait
    ("Too many sync wait commands"), but Tile freely attaches several. Hoist
    the extra waits onto same-engine nops inserted right before the
    instruction — engine streams are in-order, so blocking semantics are
    identical."""
    cnt = 0
    for fn in nc.m.functions:
        for bb in fn.blocks:
            new_insts = []
            for inst in bb.instructions:
                si = inst.sync_info
                waits = list(si.on_wait) if si is not None else []
                if len(waits) > maxw:
                    for i in range(0, len(waits) - maxw, maxw):
                        nop = mybir.InstNoOp(name=f"syncsplit-{cnt}")
                        cnt += 1
                        nop.engine = inst.engine
                        nop.sync_info = mybir.SyncInfo(
                            on_wait=waits[i : i + maxw], on_update=[]
                        )
                        new_insts.append(nop)
                    si.on_wait = waits[len(waits) - maxw :]
                new_insts.append(inst)
            if len(new_insts) != len(bb.instructions):
                bb.instructions = new_insts
    return cnt


def build_nc() -> bass.Bass:
    nc = bass.Bass()

    xT8 = nc.dram_tensor("xT8", [D, B], F8, kind="ExternalInput")
    hT8 = nc.dram_tensor("hT8", [U, B], F8, kind="ExternalInput")
    xTb = nc.dram_tensor("xTb", [D, B], BF16, kind="ExternalInput")
    hTb = nc.dram_tensor("hTb", [U, B], BF16, kind="ExternalInput")
    # fp8 weights: gates [f, i] only, 64x pre-scale   [D, 512]
    wx8 = nc.dram_tensor("wx8", [D, 2 * US], F8, kind="ExternalInput")
    wh8 = nc.dram_tensor("wh8", [U, 2 * US], F8, kind="ExternalInput")
    # bf16 weights: all gates [f,i,o,g], 64x pre-scale [D, 1024]; only the
    # o,g columns and f's k-tail rows are ever read
    wxb = nc.dram_tensor("wxb", [D, 4 * US], BF16, kind="ExternalInput")
    whb = nc.dram_tensor("whb", [U, 4 * US], BF16, kind="ExternalInput")
    # bias, host-prepped to [128, 8]: column j = units [j*128,(j+1)*128) of
    # the concatenated [f,i,o,g] 1024-unit block (gate j//2, unit-tile j%2)
    bias = nc.dram_tensor("bias", [128, 4 * UT], F32, kind="ExternalInput")
    cT = nc.dram_tensor("cT", [US, B], F32, kind="ExternalInput")
    h_newT = nc.dram_tensor("h_newT", [US, B], F32, kind="ExternalOutput")
    c_newT = nc.dram_tensor("c_newT", [US, B], F32, kind="ExternalOutput")

    wx8_r = wx8.rearrange("(kp t p) u -> p kp t u", p=128, t=2)  # [128,8,2,512]
    wh8_r = wh8.rearrange("(kp t p) u -> p kp t u", p=128, t=2)
    wxb_r = wxb.rearrange("(kt p) u -> p kt u", p=128)           # [128,16,1024]
    whb_r = whb.rearrange("(kt p) u -> p kt u", p=128)
    xT8_r = xT8.rearrange("(kt p) b -> p kt b", p=128)           # [128,16,B]
    hT8_r = hT8.rearrange("(kt p) b -> p kt b", p=128)
    xTb_r = xTb.rearrange("(kt p) b -> p kt b", p=128)
    hTb_r = hTb.rearrange("(kt p) b -> p kt b", p=128)

    with tile.TileContext(nc) as tc:
        with (
            tc.tile_pool(name="wpool", bufs=1) as wpool,
            tc.tile_pool(name="singles", bufs=1) as singles,
            tc.tile_pool(name="acts", bufs=2) as apool,
            tc.tile_pool(name="ew", bufs=3) as epool,
            tc.tile_pool(name="psum", bufs=8, space="PSUM") as ppool,
        ):
            nsl0 = bass.ts(0, NB)
            # ---- startup stream (single FIFO ring), ordered to track the
            # n=0 k-outer compute: fp8 acts+weights first (gates i,f), then
            # the bf16 side (gates o,g and f's k-tail).
            # n=0 activations arrive chunked; the chunk tiles share the
            # steady-state pool tags, so the bufs=2 rotation both bounds SBUF
            # and threads the n=1 reuse dependencies. The first x/weight
            # chunks are small so the first DoubleRow matmul starts early;
            # weights use separate tiles per chunk for fine-grained deps.
            x80c = []
            for k0, k1 in ((0, 4), (4, 8), (8, 16)):
                # first chunk gets a dedicated slot so the shared x8_sb tag
                # sees only 2 n=0 allocations (no FIFO-stalling WAR reuse)
                tag = "x80a" if k0 == 0 else "x8_sb"
                t = apool.tile([128, k1 - k0, NB], F8, tag=tag, bufs=1 if k0 == 0 else None)
                nc.sync.dma_start(out=t[:], in_=xT8_r[:, k0:k1, nsl0])
                x80c.append((k0, t))
                if k0 == 0:
                    wx8_a = wpool.tile([128, 2, 2, 2 * US], F8)
                    nc.sync.dma_start(out=wx8_a[:], in_=wx8_r[:, 0:2])
                if k0 == 4:
                    wx8_b = wpool.tile([128, PAIRS - 2, 2, 2 * US], F8)
                    nc.sync.dma_start(out=wx8_b[:], in_=wx8_r[:, 2:])

            def x80(j):
                ci = min(j // 2, 2)
                k0, t = x80c[ci]
                return t[:, 2 * j - k0 : 2 * j - k0 + 2, :]
            h80c = []
            xb0 = []
            wxb_og_c = []

            def h8_chunk(jc):
                t = apool.tile([128, 8, NB], F8, tag="h8_sb", name=f"h80c{jc}")
                nc.sync.dma_start(out=t[:], in_=hT8_r[:, 8 * jc : 8 * jc + 8, nsl0])
                h80c.append(t)
                wt = wpool.tile([128, PAIRS // 2, 2, 2 * US], F8, tag=f"wh8{jc}")
                nc.sync.dma_start(out=wt[:], in_=wh8_r[:, 4 * jc : 4 * jc + 4])
                return wt

            def xb_chunk(jc):
                t = apool.tile([128, 8, NB], BF16, tag="xb_sb", name=f"xb0c{jc}")
                nc.sync.dma_start(out=t[:], in_=xTb_r[:, 8 * jc : 8 * jc + 8, nsl0])
                xb0.append(t)
                wt = wpool.tile([128, 8, 2 * US], BF16, tag=f"wxog{jc}")
                nc.sync.dma_start(out=wt[:], in_=wxb_r[:, 8 * jc : 8 * jc + 8, 2 * US :])
                wxb_og_c.append(wt)

            wh8_a = h8_chunk(0)
            xb_chunk(0)
            wh8_b = h8_chunk(1)
            b_sb = singles.tile([128, 4 * UT], F32)
            nc.sync.dma_start(out=b_sb[:], in_=bias[:])
            xb_chunk(1)
            h80 = lambda j: h80c[j // 4][:, 2 * j - 8 * (j // 4) : 2 * j - 8 * (j // 4) + 2, :]
            hb0 = []
            whb_og_c = []
            for jc in range(2):
                t = apool.tile([128, 8, NB], BF16, tag="hb_sb")
                nc.sync.dma_start(out=t[:], in_=hTb_r[:, 8 * jc : 8 * jc + 8, nsl0])
                hb0.append(t)
                wt = wpool.tile([128, 8, 2 * US], BF16, tag=f"whog{jc}")
                nc.sync.dma_start(out=wt[:], in_=whb_r[:, 8 * jc : 8 * jc + 8, 2 * US :])
                whb_og_c.append(wt)

            def act_gate(ps, gi, ut, name):
                g_sb = epool.tile([128, NB], F32, tag=f"gate{gi}", name=name)
                nc.scalar.activation(
                    g_sb[:],
                    ps[:],
                    AF.Tanh if gi == 3 else AF.Sigmoid,
                    bias=b_sb[:, gi * UT + ut : gi * UT + ut + 1],
                    scale=1.0 / W_SCALE,
                )
                return g_sb

            def elementwise(pss, n, ut):
                # pss indexed by gate order [f, i, o, g]; consume gates in
                # completion order g, i, f, o
                nsl = bass.ts(n, NB)
                usl = slice(ut * 128, (ut + 1) * 128)
                c_sb = epool.tile([128, NB], F32, tag="c_sb", name="c_sb")
                nc.sync.dma_start(out=c_sb[:], in_=cT[usl, nsl])
                g_t = act_gate(pss[3], 3, ut, "g_t")
                i_t = act_gate(pss[1], 1, ut, "i_t")
                nc.vector.tensor_mul(i_t[:], i_t[:], g_t[:])      # i*g
                f_t = act_gate(pss[0], 0, ut, "f_t")
                nc.vector.tensor_mul(f_t[:], f_t[:], c_sb[:])     # f*c
                cn = epool.tile([128, NB], F32, tag="cn", name="cn")
                nc.vector.tensor_add(cn[:], f_t[:], i_t[:])       # c_new
                nc.sync.dma_start(out=c_newT[usl, nsl], in_=cn[:])
                nc.scalar.activation(g_t[:], cn[:], AF.Tanh)      # tanh(c_new)
                o_t = act_gate(pss[2], 2, ut, "o_t")
                nc.vector.tensor_mul(o_t[:], o_t[:], g_t[:])      # h_new
                nc.sync.dma_start(out=h_newT[usl, nsl], in_=o_t[:])

            # weight AP accessors over the chunked tiles; column base:
            # fp8 tiles hold gates [f, i], og tiles hold gates [o, g]
            def wx8_ap(j, gi, ut):
                t, jj = (wx8_a, j) if j < 2 else (wx8_b, j - 2)
                c = gi * US + ut * 128
                return t[:, jj, :, c : c + 128]

            def wh8_ap(j, gi, ut):
                t, jj = (wh8_a, j) if j < 4 else (wh8_b, j - 4)
                c = gi * US + ut * 128
                return t[:, jj, :, c : c + 128]

            def wxog_ap(kt, gi, ut):
                c = (gi - 2) * US + ut * 128
                return wxb_og_c[kt // 8][:, kt % 8, c : c + 128]

            def whog_ap(kt, gi, ut):
                c = (gi - 2) * US + ut * 128
                return whb_og_c[kt // 8][:, kt % 8, c : c + 128]

            # ---- matmul emitters (shared between n=0 k-outer and n>=1) ----
            def mm_f8(ps, w_ap, mv, start, stop):
                nc.tensor.matmul(
                    ps[:], w_ap, mv, start=start, stop=stop, perf_mode=DR
                )

            def mm_og(ps, w_ap, mv, start, stop):
                nc.tensor.matmul(ps[:], w_ap, mv, start=start, stop=stop)

            # --- n = 0: k-outer, ordered to track the DMA arrival stream.
            # Groups: (gate, ut) -> PSUM bank; i,f consume fp8 pairs first,
            # then o,g (+ f tail) consume the bf16 stream.
            ps0 = [
                [
                    ppool.tile([128, NB], F32, tag="ps", name=f"ps{ut}{gi}")
                    for gi in range(4)
                ]
                for ut in range(UT)
            ]
            # fp8 x pairs (i then f per pair so i leads)
            for j in range(PAIRS):
                mv = x80(j)
                for ut in range(UT):
                    mm_f8(ps0[ut][1], wx8_ap(j, 1, ut), mv, start=(j == 0), stop=False)
                    mm_f8(ps0[ut][0], wx8_ap(j, 0, ut), mv, start=(j == 0), stop=False)
            # fp8 h pairs
            for j in range(PAIRS):
                mv = h80(j)
                for ut in range(UT):
                    mm_f8(ps0[ut][1], wh8_ap(j, 1, ut), mv, start=False, stop=(j == PAIRS - 1))
                    mm_f8(ps0[ut][0], wh8_ap(j, 0, ut), mv, start=False, stop=(j == PAIRS - 1))
            # bf16 x k-tiles: o,g (+ f tail rows)
            for kt in range(KX):
                mv = xb0[kt // 8][:, kt % 8, :]
                for ut in range(UT):
                    for gi in (3, 2):
                        mm_og(ps0[ut][gi], wxog_ap(kt, gi, ut), mv, start=(kt == 0), stop=False)
            # bf16 h k-tiles
            for kt in range(KX):
                mv = hb0[kt // 8][:, kt % 8, :]
                for ut in range(UT):
                    for gi in (3, 2):
                        mm_og(ps0[ut][gi], whog_ap(kt, gi, ut), mv, start=False, stop=(kt == KX - 1))
            for ut in range(UT):
                elementwise(ps0[ut], 0, ut)

            # --- n = 1..7: gate-outer in consumption order (g, i, f, o)
            for n in range(1, NT):
                nsl = bass.ts(n, NB)
                x8_sb = apool.tile([128, KX, NB], F8, tag="x8_sb")
                nc.sync.dma_start(out=x8_sb[:], in_=xT8_r[:, :, nsl])
                h8_sb = apool.tile([128, KX, NB], F8, tag="h8_sb")
                nc.sync.dma_start(out=h8_sb[:], in_=hT8_r[:, :, nsl])
                xb_sb = apool.tile([128, KX, NB], BF16, tag="xb_sb")
                nc.sync.dma_start(out=xb_sb[:], in_=xTb_r[:, :, nsl])
                hb_sb = apool.tile([128, KX, NB], BF16, tag="hb_sb")
                nc.sync.dma_start(out=hb_sb[:], in_=hTb_r[:, :, nsl])

                for ut in range(UT):
                    pss = [
                        ppool.tile([128, NB], F32, tag="ps", name=f"ps{gi}")
                        for gi in range(4)
                    ]
                    # gate i first: its fp8 x tile is the iteration's
                    # earliest-arriving input
                    for j in range(PAIRS):
                        mm_f8(pss[1], wx8_ap(j, 1, ut), x8_sb[:, 2 * j : 2 * j + 2, :],
                              start=(j == 0), stop=False)
                    for j in range(PAIRS):
                        mm_f8(pss[1], wh8_ap(j, 1, ut), h8_sb[:, 2 * j : 2 * j + 2, :],
                              start=False, stop=(j == PAIRS - 1))
                    # gate g: all bf16
                    for kt in range(KX):
                        mm_og(pss[3], wxog_ap(kt, 3, ut), xb_sb[:, kt, :],
                              start=(kt == 0), stop=False)
                    for kt in range(KX):
                        mm_og(pss[3], whog_ap(kt, 3, ut), hb_sb[:, kt, :],
                              start=False, stop=(kt == KX - 1))
                    # gate f: all fp8 pairs
                    for j in range(PAIRS):
                        mm_f8(pss[0], wx8_ap(j, 0, ut), x8_sb[:, 2 * j : 2 * j + 2, :],
                              start=(j == 0), stop=False)
                    for j in range(PAIRS):
                        mm_f8(pss[0], wh8_ap(j, 0, ut), h8_sb[:, 2 * j : 2 * j + 2, :],
                              start=False, stop=(j == PAIRS - 1))
                    # gate o: all bf16
                    for kt in range(KX):
                        mm_og(pss[2], wxog_ap(kt, 2, ut), xb_sb[:, kt, :],
                              start=(kt == 0), stop=False)
                    for kt in range(KX):
                        mm_og(pss[2], whog_ap(kt, 2, ut), hb_sb[:, kt, :],
                              start=False, stop=(kt == KX - 1))
                    elementwise(pss, n, ut)
    _split_excess_waits(nc)
    return nc


_NC_CACHE = None


def _get_nc():
    global _NC_CACHE
    if _NC_CACHE is None:
        _NC_CACHE = build_nc()
    return _NC_CACHE


def make_in_maps(x, h, c, Wxf, Wxi, Wxo, Wxg, bf, bi, bo, bg, Whf, Whi, Who, Whg):
    f8 = ml_dtypes.float8_e4m3
    bf16 = ml_dtypes.bfloat16
    x = np.asarray(x, np.float32)
    h = np.asarray(h, np.float32)
    xT = np.ascontiguousarray(x.T)
    hT = np.ascontiguousarray(h.T)
    xT8 = xT.astype(f8)
    hT8 = hT.astype(f8)
    xTb = xT.astype(bf16)
    hTb = hT.astype(bf16)
    c = np.asarray(c, np.float32)
    Wx = np.stack([np.asarray(w, np.float32) for w in (Wxf, Wxi, Wxo, Wxg)])
    Wh = np.stack([np.asarray(w, np.float32) for w in (Whf, Whi, Who, Whg)])
    bias = np.stack([np.asarray(v, np.float32) for v in (bf, bi, bo, bg)])

    in_maps = []
    for i in range(N_CORES):
        s = slice(i * US, (i + 1) * US)
        wx8_i = np.concatenate(
            [Wx[g, :, s] * W_SCALE for g in range(2)], axis=1
        ).astype(f8)
        wh8_i = np.concatenate(
            [Wh[g, :, s] * W_SCALE for g in range(2)], axis=1
        ).astype(f8)
        wxb_i = np.concatenate(
            [Wx[g, :, s] * W_SCALE for g in range(4)], axis=1
        ).astype(bf16)
        whb_i = np.concatenate(
            [Wh[g, :, s] * W_SCALE for g in range(4)], axis=1
        ).astype(bf16)
        b_i = np.concatenate([bias[g, s] for g in range(4)])  # [1024]
        b_i = np.ascontiguousarray(b_i.reshape(4 * UT, 128).T)  # [128, 8]
        cT_i = np.ascontiguousarray(c[:, s].T)  # [US, B]
        in_maps.append(
            {
                "xT8": xT8, "hT8": hT8, "xTb": xTb, "hTb": hTb,
                "wx8": wx8_i, "wh8": wh8_i, "wxb": wxb_i, "whb": whb_i,
                "bias": b_i, "cT": cT_i,
            }
        )
    return in_maps


def run(in_maps, **kwargs):
    nc = _get_nc()
    return run_bass_kernel_spmd(nc, in_maps, list(range(N_CORES)), **kwargs)


def gather(results):
    h_new = np.empty((B, U), np.float32)
    c_new = np.empty((B, U), np.float32)
    for i in range(N_CORES):
        s = slice(i * US, (i + 1) * US)
        h_new[:, s] = results[i]["h_newT"].T
        c_new[:, s] = results[i]["c_newT"].T
    return h_new, c_new


def kernel(**inputs):
    res = run(make_in_maps(**inputs))
    return gather(res.results)
